# revision 1
# baseline (speedup 1.0000x reference)
"""Trainium2 Bass kernel for nn_CrossAttention (B=2, S=2048, D=1024, H=16).

Sharding: 8 cores = 2 batches x 4 head-groups (4 heads each). Because the
reference does a raw-memory reshape of the (B,H,Sq,dh)-contiguous attention
output to (B,Sq,D), head h's output maps to the contiguous output rows
[h*128, (h+1)*128) — so each core independently produces a [512, 1024]
block of the final output; no cross-core communication is needed.

Per-core device kernel (all fp32 / fp32r):
  1. Q/K/V projections from host-pretransposed decT/encT (contraction dim on
     partitions). qT/kT stored [dh_group, S]; v stored natural [S, dh] with a
     ones column appended per head (softmax denominator rides the PV matmul).
  2. S^T = k^T . q per head in PSUM (K=64, head pairs at partitions 0-63 /
     64-127 for row-group concurrency), exp on ACT reading 2-bank PSUM gulps.
  3. PV: z^T[dh+1, sq] accumulated over sk chunks; row 64 = sum(exp).
  4. PE-transpose back to natural [sq, dh+1], scale by 1/L, reassemble the
     raw-reshape layout via an HBM bounce, add residual, LayerNorm, store.
"""

import numpy as np

import concourse.bass as bass
import concourse.tile as tile
from concourse import bacc, mybir
from concourse.bass_utils import run_bass_kernel_spmd
from concourse.masks import make_identity

F32 = mybir.dt.float32
F32R = mybir.dt.float32r
AF = mybir.ActivationFunctionType
OP = mybir.AluOpType

B = 2
S = 2048          # sequence length (q and k)
D = 1024          # d_model
DH = 64           # head dim
HPC = 4           # heads per core
DG = HPC * DH     # 256 projection out-dims per core
ROWS = HPC * 128  # 512 output rows per core
N_CORES = 8
LN_EPS = 1e-5


def build_bass(reps=1):
    nc = bacc.Bacc(None, target_bir_lowering=False, debug=False)

    encT_h = nc.declare_dram_parameter("encT", [D, S], F32R, isOutput=False)
    decT_h = nc.declare_dram_parameter("decT", [D, S], F32R, isOutput=False)
    wqT_h = nc.declare_dram_parameter("wqT", [D, DG], F32R, isOutput=False)
    wkT_h = nc.declare_dram_parameter("wkT", [D, DG], F32R, isOutput=False)
    wvT_h = nc.declare_dram_parameter("wvT", [D, DG], F32R, isOutput=False)
    bq_h = nc.declare_dram_parameter("bq", [DG], F32, isOutput=False)
    bk_h = nc.declare_dram_parameter("bk", [DG], F32, isOutput=False)
    bv_h = nc.declare_dram_parameter("bv", [DG], F32, isOutput=False)
    dec_blk_h = nc.declare_dram_parameter("dec_blk", [ROWS, D], F32, isOutput=False)
    gamma_h = nc.declare_dram_parameter("gamma", [D], F32, isOutput=False)
    beta_h = nc.declare_dram_parameter("beta", [D], F32, isOutput=False)
    out_h = nc.declare_dram_parameter("out", [ROWS, D], F32, isOutput=True)

    def bcast(ap, p=128):
        return bass.AP(tensor=ap.tensor, offset=ap.offset, ap=[[0, p]] + list(ap.ap))

    with tile.TileContext(nc) as tc:
        with (
            tc.tile_pool(name="consts", bufs=1) as consts,
            tc.tile_pool(name="w", bufs=1) as wpool,
            tc.tile_pool(name="kq", bufs=1) as kq,
            tc.tile_pool(name="vp", bufs=16) as vpool,
            tc.tile_pool(name="dram", bufs=1, space="DRAM") as dram,
        ):
            # kT/qT: [2 tiles][128, S]; tile m holds heads 2m (parts 0:64), 2m+1 (64:128)
            kT = [kq.tile([128, S], F32R, name=f"kT{m}", tag=f"kT{m}") for m in range(2)]
            qT = [kq.tile([128, S], F32R, name=f"qT{m}", tag=f"qT{m}") for m in range(2)]
            zd = dram.tile([HPC, S, DH], F32)

            for _rep in range(reps):
                v_sb = []
                # ---------------- Phase A: projections ----------------
                with tc.tile_pool(name="xt", bufs=14) as xt:
                    # encT (gpsimd queue) and decT (SP queue) load in parallel
                    et, dt_ = [], []
                    for k in range(6):
                        t = xt.tile([128, S], F32R, name="et", tag="xt")
                        nc.gpsimd.dma_start(out=t, in_=encT_h[k * 128:(k + 1) * 128, :])
                        et.append(t)
                    for k in range(6):
                        t = xt.tile([128, S], F32R, name="dt", tag="xt")
                        nc.sync.dma_start(out=t, in_=decT_h[k * 128:(k + 1) * 128, :])
                        dt_.append(t)
                    if _rep == 0:
                        wk_sb = wpool.tile([128, 8, DG], F32R)
                        nc.scalar.dma_start(out=wk_sb, in_=wkT_h[:].rearrange("(t p) n -> p t n", p=128))
                        wv_sb = wpool.tile([128, 8, DG], F32R)
                        nc.scalar.dma_start(out=wv_sb, in_=wvT_h[:].rearrange("(t p) n -> p t n", p=128))
                        wq_sb = wpool.tile([128, 8, DG], F32R)
                        nc.scalar.dma_start(out=wq_sb, in_=wqT_h[:].rearrange("(t p) n -> p t n", p=128))
                    for k in range(6, 8):
                        t = xt.tile([128, S], F32R, name="et", tag="xt")
                        nc.scalar.dma_start(out=t, in_=encT_h[k * 128:(k + 1) * 128, :])
                        et.append(t)
                    for k in range(6, 8):
                        t = xt.tile([128, S], F32R, name="dt", tag="xt")
                        nc.scalar.dma_start(out=t, in_=decT_h[k * 128:(k + 1) * 128, :])
                        dt_.append(t)
                    if _rep == 0:
                        bq_sb = consts.tile([128, 2], F32)
                        nc.scalar.dma_start(out=bq_sb, in_=bq_h[:].rearrange("(t p) -> p t", p=128))
                        bk_sb = consts.tile([128, 2], F32)
                        nc.scalar.dma_start(out=bk_sb, in_=bk_h[:].rearrange("(t p) -> p t", p=128))
                        bv_b = consts.tile([128, DG], F32)
                        nc.scalar.dma_start(out=bv_b, in_=bcast(bv_h[:]))
                        gamma_b = consts.tile([128, D], F32)
                        nc.scalar.dma_start(out=gamma_b, in_=bcast(gamma_h[:]))
                        beta_b = consts.tile([128, D], F32)
                        nc.scalar.dma_start(out=beta_b, in_=bcast(beta_h[:]))
                        eps_sb = consts.tile([128, 1], F32)
                        nc.vector.memset(eps_sb, LN_EPS)
                        warm = consts.tile([128, 1], F32)
                        nc.scalar.activation(out=warm, in_=eps_sb, func=AF.Exp)
                        ones_c = consts.tile([128, 1], F32)
                        nc.vector.memset(ones_c, 1.0)
                        ident = consts.tile([128, 128], F32)
                        make_identity(nc, ident)
                    # K projection: k-outer over 8 live psums, streams as encT arrives
                    with tc.tile_pool(name="pk8", bufs=1, space="PSUM") as pk8:
                        kps = [pk8.tile([128, 512], F32, name=f"kps{mn}", tag=f"kps{mn}")
                               for mn in range(8)]
                        for k in range(8):
                            for mn in range(8):
                                m, n = mn // 4, mn % 4
                                nc.tensor.matmul(
                                    kps[mn],
                                    lhsT=wk_sb[:, k, m * 128:(m + 1) * 128],
                                    rhs=et[k][:, n * 512:(n + 1) * 512],
                                    start=(k == 0),
                                    stop=(k == 7),
                                    skip_group_check=True,
                                )
                        for mn in range(8):
                            m, n = mn // 4, mn % 4
                            nc.vector.tensor_scalar_add(
                                kT[m][:, n * 512:(n + 1) * 512], kps[mn], bk_sb[:, m:m + 1]
                            )
                    # Q projection: k-outer over 8 live psums
                    with tc.tile_pool(name="pq8", bufs=1, space="PSUM") as pq8:
                        qps = [pq8.tile([128, 512], F32, name=f"qps{mn}", tag=f"qps{mn}")
                               for mn in range(8)]
                        for k in range(8):
                            for mn in range(8):
                                m, n = mn // 4, mn % 4
                                nc.tensor.matmul(
                                    qps[mn],
                                    lhsT=wq_sb[:, k, m * 128:(m + 1) * 128],
                                    rhs=dt_[k][:, n * 512:(n + 1) * 512],
                                    start=(k == 0),
                                    stop=(k == 7),
                                    skip_group_check=True,
                                )
                        for mn in range(8):
                            m, n = mn // 4, mn % 4
                            nc.vector.tensor_scalar_add(
                                qT[m][:, n * 512:(n + 1) * 512], qps[mn], bq_sb[:, m:m + 1]
                            )

                    # V projection
                    with tc.tile_pool(name="pjv", bufs=2, space="PSUM") as pjv:
                        for i in range(16):
                            ps = pjv.tile([128, DG], F32, tag="pv")
                            for k in range(8):
                                nc.tensor.matmul(
                                    ps,
                                    lhsT=et[k][:, i * 128:(i + 1) * 128],
                                    rhs=wv_sb[:, k, :],
                                    start=(k == 0),
                                    stop=(k == 7),
                                )
                            vt = vpool.tile([128, HPC * 65], F32R, tag="v")
                            oc = ones_c[:]
                            oc4 = bass.AP(tensor=oc.tensor, offset=oc.offset, ap=[list(oc.ap[0]), [0, 4]])
                            nc.vector.tensor_copy(vt[:, 64:HPC * 65:65], oc4)
                            for hl in range(HPC):
                                nc.vector.tensor_add(
                                    vt[:, hl * 65:hl * 65 + 64],
                                    ps[:, hl * 64:(hl + 1) * 64],
                                    bv_b[:, hl * 64:(hl + 1) * 64],
                                )
                            v_sb.append(vt)
                # ---------------- Phase B: attention ----------------
                with (
                    tc.tile_pool(name="sps", bufs=2, space="PSUM") as sps,
                    tc.tile_pool(name="pvps", bufs=2, space="PSUM") as pvps,
                    tc.tile_pool(name="tps", bufs=2, space="PSUM") as tps,
                    tc.tile_pool(name="exps", bufs=6) as exps,
                    tc.tile_pool(name="ztp", bufs=3) as ztp,
                    tc.tile_pool(name="zhp", bufs=4) as zhp,
                    tc.tile_pool(name="xp", bufs=2) as xp,
                    tc.tile_pool(name="sm", bufs=8) as sm,
                ):
                    for hp in range(2):
                        zhs = [zhp.tile([128, D], F32, name="zh", tag="zh") for _ in range(2)]
                        for j in range(4):
                            pvs = [pvps.tile([65, 512], F32, name="pv", tag="pv") for _ in range(2)]
                            for g in range(16):
                                sp = sps.tile([128, 2, 512], F32, tag="s")
                                for hl in range(2):
                                    nc.tensor.matmul(
                                        sp[:, hl, :],
                                        lhsT=kT[hp][hl * 64:hl * 64 + 64, g * 128:(g + 1) * 128],
                                        rhs=qT[hp][hl * 64:hl * 64 + 64, j * 512:(j + 1) * 512],
                                        start=True,
                                        stop=True,
                                    )
                                ex = exps.tile([128, 2, 512], F32R, tag="ex")
                                nc.scalar.activation(out=ex, in_=sp, func=AF.Exp)
                                for hl in range(2):
                                    nc.tensor.matmul(
                                        pvs[hl],
                                        lhsT=v_sb[g][:, (hp * 2 + hl) * 65:(hp * 2 + hl) * 65 + 65],
                                        rhs=ex[:, hl, :],
                                        start=(g == 0),
                                        stop=(g == 15),
                                        skip_group_check=True,
                                    )
                            for hl in range(2):
                                zt_t = ztp.tile([65, 512], F32, tag="zt")
                                nc.vector.tensor_copy(zt_t, pvs[hl])
                                for q in range(4):
                                    tp = tps.tile([128, 65], F32, tag="tp")
                                    nc.tensor.transpose(
                                        out=tp, in_=zt_t[:, q * 128:(q + 1) * 128],
                                        identity=ident[0:65, 0:65],
                                    )
                                    rc = sm.tile([128, 1], F32, tag="rc")
                                    nc.vector.reciprocal(rc, tp[:, 64:65])
                                    nc.vector.tensor_scalar_mul(
                                        zhs[hl][:, (j * 4 + q) * 64:(j * 4 + q + 1) * 64],
                                        tp[:, 0:64],
                                        rc,
                                    )
                                h = hp * 2 + hl
                                nc.gpsimd.dma_start(
                                    out=zd[h][j * 512:(j + 1) * 512, :].rearrange(
                                        "(q p) d -> p q d", p=128),
                                    in_=zhs[hl][:, j * 256:(j + 1) * 256].rearrange(
                                        "p (q d) -> p q d", d=DH),
                                )

                        # ------ Phase C for this head pair (no ACT table switch) --
                        for h in (hp * 2, hp * 2 + 1):
                            x_t = xp.tile([128, D], F32, tag="x")
                            nc.gpsimd.dma_start(out=x_t, in_=zd[h].rearrange("(u w) d -> u (w d)", w=16))
                            dct = xp.tile([128, D], F32, tag="dc")
                            nc.gpsimd.dma_start(out=dct, in_=dec_blk_h[h * 128:(h + 1) * 128, :])
                            nc.vector.tensor_add(x_t, x_t, dct)
                            st = sm.tile([128, 2, 6], F32, tag="st")
                            for s2 in range(2):
                                nc.vector.bn_stats(out=st[:, s2, :], in_=x_t[:, s2 * 512:(s2 + 1) * 512])
                            mv = sm.tile([128, 2], F32, tag="mv")
                            nc.vector.bn_aggr(out=mv, in_=st)
                            # rstd = 1/sqrt(var+eps) via bit-trick seed + 2 Newton steps (DVE only)
                            vv = sm.tile([128, 1], F32, tag="vv")
                            nc.vector.tensor_scalar_add(vv, mv[:, 1:2], LN_EPS)
                            ti = sm.tile([128, 1], mybir.dt.int32, tag="ti")
                            nc.vector.tensor_scalar(
                                out=ti, in0=vv[:].bitcast(mybir.dt.int32), scalar1=1,
                                scalar2=None, op0=OP.logical_shift_right,
                            )
                            nc.vector.tensor_scalar(
                                out=ti, in0=ti, scalar1=-1, scalar2=0x5F3759DF,
                                op0=OP.mult, op1=OP.add,
                            )
                            y = sm.tile([128, 1], F32, tag="y")
                            nc.vector.tensor_copy(y, ti[:].bitcast(F32))
                            t1 = sm.tile([128, 1], F32, tag="t1")
                            for _ in range(2):
                                nc.vector.tensor_mul(t1, vv, y)
                                nc.vector.tensor_mul(t1, t1, y)
                                nc.vector.tensor_scalar(
                                    out=t1, in0=t1, scalar1=-0.5, scalar2=1.5,
                                    op0=OP.mult, op1=OP.add,
                                )
                                nc.vector.tensor_mul(y, y, t1)
                            rstd = y
                            xn = xp.tile([128, D], F32, tag="xn")
                            nc.vector.tensor_scalar(
                                out=xn, in0=x_t, scalar1=mv[:, 0:1], scalar2=rstd,
                                op0=OP.subtract, op1=OP.mult,
                            )
                            nc.vector.tensor_mul(xn, xn, gamma_b)
                            nc.vector.tensor_add(xn, xn, beta_b)
                            nc.gpsimd.dma_start(out=out_h[h * 128:(h + 1) * 128, :], in_=xn)

    nc.compile()
    return nc


_NC_CACHE = None


def _get_nc():
    global _NC_CACHE
    if _NC_CACHE is None:
        _NC_CACHE = build_bass()
    return _NC_CACHE


def make_in_maps(encoded, decoded, Wq, bq, Wk, bk, Wv, bv, gamma, beta):
    encoded = np.asarray(encoded, dtype=np.float32)
    decoded = np.asarray(decoded, dtype=np.float32)
    Wq, bq = np.asarray(Wq, np.float32), np.asarray(bq, np.float32)
    Wk, bk = np.asarray(Wk, np.float32), np.asarray(bk, np.float32)
    Wv, bv = np.asarray(Wv, np.float32), np.asarray(bv, np.float32)
    gamma, beta = np.asarray(gamma, np.float32), np.asarray(beta, np.float32)

    encT = [np.ascontiguousarray(encoded[b].T) for b in range(B)]
    decT = [np.ascontiguousarray(decoded[b].T) for b in range(B)]
    in_maps = []
    for c in range(N_CORES):
        b, hg = c // HPC, c % HPC
        sl = slice(hg * DG, (hg + 1) * DG)
        rows = slice(hg * ROWS, (hg + 1) * ROWS)
        in_maps.append({
            "encT": encT[b],
            "decT": decT[b],
            "wqT": np.ascontiguousarray(Wq[sl, :].T),
            "wkT": np.ascontiguousarray(Wk[sl, :].T),
            "wvT": np.ascontiguousarray(Wv[sl, :].T),
            "bq": np.ascontiguousarray(bq[sl]),
            "bk": np.ascontiguousarray(bk[sl]),
            "bv": np.ascontiguousarray(bv[sl]),
            "dec_blk": np.ascontiguousarray(decoded[b, rows]),
            "gamma": gamma,
            "beta": beta,
        })
    return in_maps


def kernel(**inputs) -> np.ndarray:
    nc = _get_nc()
    in_maps = make_in_maps(**inputs)
    res = run_bass_kernel_spmd(nc, in_maps, list(range(N_CORES)))
    out = np.empty((B, S, D), dtype=np.float32)
    for c in range(N_CORES):
        b, hg = c // HPC, c % HPC
        out[b, hg * ROWS:(hg + 1) * ROWS, :] = res.results[c]["out"]
    return out

